# revision 22
# baseline (speedup 1.0000x reference)
"""Trainium2 Bass kernel for nn_Encoder_21964462752332 (parity-plane rewrite).

Math: the swap-test circuit per 4x4 patch p reduces to
    out = 0.5 + 0.5 * ||A p||^2 / ||p||^2 = (num + den) / (2 den),
with A = U[:4, :], num = ||A p||^2, den = ||p||^2 (U = 16x16 MPS orthogonal
matrix built from the 12 weights_mps floats; see _build_U).

Dataflow (per core, 32 images, SPMD over 8 cores):
  The stride-2 / kernel-4 patch extraction is re-expressed over the four
  image parity planes Pl[pi,pj][r,c] = img[2r+pi, 2c+pj] (32x32 each).
  Patch (oh,ow) tap (kh,kw) = Pl[kh%2,kw%2][oh+kh//2, ow+kw//2], so with a
  [128, 1056] planes tile (partition = image*4 + plane, col = 32*r + c,
  pixel grid padded to 32 cols incl. a garbage ow=31 so every shifted view
  is a CONTIGUOUS column range):
    q        : 4 shift-matmuls, blockdiag(A-slice) weights -> PSUM (2 col
               chunks of <=512, one PSUM bank each)
    q^2      : ACT Square PSUM->SBUF (bf16)
    planes^2 : DVE tensor_tensor (bf16, 2x mode)
    num,
    2*den    : ones-weight matmuls packed 4-way into 128 partitions by
               column strips (tile_position=(0,32c)): zzn/zzd[32c+i, k] =
               image i, pixel-buffer col 248c+k.  den = 4 shift-matmuls
               over planes^2; num = 1 matmul over q^2 per strip.
    out      : num/(2den): ACT copies PSUM->SBUF (den early, num late),
               DVE reciprocal of den hidden under the num matmuls, then
               one DVE multiply -> bf16.  Host adds the +0.5.
  Input ships once as raw bf16 pixels (~0.3 MB/core vs 2.1 MB im2col f32
  before), split into 3 DMAs ordered by first use (shift-0 weights + first
  plane rows first).  All matmuls bf16 (1 cyc/row).  Output ships bf16
  packed [128, 256].  Warm-up matmuls on an uninitialized tile keep the PE
  busy from ~t=0 so the p-state ramp (full clock after ~3us of activity)
  completes before the real matmuls; this also warms the real HAM.
  Known toolchain constraints baked in: DVE cannot touch PSUM (ISA check),
  TT-divide is not a valid DVE instruction, SBUF TT operands must share a
  base partition, and a tile needs >=1 writer to be allocated.
"""

import numpy as np
import ml_dtypes

# ---- problem geometry (hardcoded per contract) ----
BS = 256
H = W = 64
OH = OW = 31
N_CORES = 8
NI = BS // N_CORES              # 32 images per core
GRID = 32 * 31                  # padded pixel grid cols (ow=31 is garbage)
PCOL0 = 576                     # planes tile offset inside P (after weights)
PCOLS = 1056                    # 1024 real plane cols + 32 pad
SHIFTS = [(0, 0), (0, 1), (1, 0), (1, 1)]
# output-row chunks (r0, nrows): N = nrows*32 <= 512 (PSUM bank); the last
# chunk is small so the post-matmul ACT/DVE/DMA tail is short
CHUNKS = [(0, 16), (16, 15)]
QW = GRID // 4                  # mm2 column-strip (quarter) width = 248
D0 = PCOL0 + 576                # first DMA: weights + planes rows 0..18

_CACHE = {}
TRACE = False            # test.py sets this to profile
TRACE_KWARGS = {}

WARM_MMS = 13            # PE ramp warm-up matmuls on zeroed SBUF
WARM_N = 256
OUT_DMAS = [(0, 512), (512, GRID)]


def _build_U(weights_mps: np.ndarray) -> np.ndarray:
    """16x16 orthogonal MPS circuit matrix; amp index bits are MSB-first in
    local data-wire order (wire 0 = most significant)."""
    Wm = np.asarray(weights_mps, dtype=np.float64)
    I2 = np.eye(2)
    CNOT = np.array(
        [[1, 0, 0, 0], [0, 1, 0, 0], [0, 0, 0, 1], [0, 0, 1, 0]], dtype=np.float64
    )

    def ry(t):
        c, s = np.cos(t / 2.0), np.sin(t / 2.0)
        return np.array([[c, -s], [s, c]])

    def emb1(U2, w):
        out = np.array([[1.0]])
        for i in range(4):
            out = np.kron(out, U2 if i == w else I2)
        return out

    def emb2(U4, w):
        return np.kron(np.eye(2 ** w), np.kron(U4, np.eye(2 ** (2 - w))))

    U = np.eye(16)
    for l in range(2):
        for b in range(3):
            U = emb1(ry(Wm[l, b, 0]), b) @ U
            U = emb1(ry(Wm[l, b, 1]), b + 1) @ U
            U = emb2(CNOT, b) @ U
    return U


def _build_bass(loop_reps=None, loop_unroll=1, empty=False):
    import concourse.bacc as bacc
    import concourse.mybir as mybir
    from concourse.tile import TileContext

    f32 = mybir.dt.float32
    bf16 = mybir.dt.bfloat16
    AF = mybir.ActivationFunctionType
    ALU = mybir.AluOpType

    nc = bacc.Bacc(None)
    blob0 = nc.dram_tensor("blob0", [128, D0], bf16, kind="ExternalInput")
    blob1 = nc.dram_tensor("blob1", [128, PCOL0 + PCOLS - D0], bf16,
                           kind="ExternalInput")
    out = nc.dram_tensor("out", [128, 256], bf16, kind="ExternalOutput")

    with TileContext(nc) as tc:
        with (
            tc.tile_pool(name="big", bufs=1) as bigpool,
            tc.tile_pool(name="work", bufs=1) as wpool,
            tc.tile_pool(name="psum", bufs=1, space="PSUM") as ppool,
        ):
            P = bigpool.tile([128, PCOL0 + PCOLS], bf16, tag="P")
            warm = bigpool.tile([128, WARM_N], bf16, tag="warm")
            wps = ppool.tile([128, WARM_N], f32, tag="wps")

            def warmup():
                # keep PE continuously busy from ~t=0 so the p-state ramp
                # (full speed after 3us) completes before the real matmuls
                # a tile must have >=1 writer to be allocated; the matmuls
                # happily consume the rest uninitialized (results discarded)
                nc.vector.memset(warm[:, 0:8], 0)
                for _ in range(WARM_MMS):
                    nc.tensor.matmul(
                        wps[:], lhsT=warm[:, 0:128], rhs=warm[:],
                        start=True, stop=True,
                    )

            def load():
                nc.sync.dma_start(out=P[:, 0:D0], in_=blob0[:, :])
                nc.sync.dma_start(out=P[:, D0:PCOL0 + PCOLS], in_=blob1[:, :])

            def body():
                qsq = wpool.tile([128, GRID], bf16, tag="qsq")
                Psq = wpool.tile([128, PCOLS], bf16, tag="Psq")
                zs = wpool.tile([128, 2 * QW], bf16, tag="zs")
                res = wpool.tile([128, 256], bf16, tag="res")
                # pad cols so the out DMA moves 512B/partition (no sub-512B
                # read-modify-write penalty); Pool is idle so memset is free
                nc.gpsimd.memset(res[:, QW:256], 0)
                # planes^2 in two pieces so chunk0's den-mms don't wait DMA1
                nc.vector.tensor_tensor(
                    Psq[:, 0:D0 - PCOL0], P[:, PCOL0:D0], P[:, PCOL0:D0],
                    ALU.mult,
                )
                nc.vector.tensor_tensor(
                    Psq[:, D0 - PCOL0:PCOLS], P[:, D0:PCOL0 + PCOLS],
                    P[:, D0:PCOL0 + PCOLS], ALU.mult,
                )

                # pass 1a (emitted first = higher scheduler priority):
                # all q matmuls + ACT squares, so the squares never queue
                # behind pass-1b/2 ACT work and the PE never stalls on them
                for ci, (r0, nr) in enumerate(CHUNKS):
                    N = nr * 32
                    c0 = r0 * 32
                    qp = ppool.tile([128, N], f32, tag=f"qp{ci}")
                    for s, (dh, dw) in enumerate(SHIFTS):
                        base = PCOL0 + (r0 + dh) * 32 + dw
                        nc.tensor.matmul(
                            qp[:], lhsT=P[:, 128 * s:128 * s + 128],
                            rhs=P[:, base:base + N],
                            start=(s == 0), stop=(s == 3),
                        )
                    nc.scalar.activation(qsq[:, c0:c0 + N], qp[:], AF.Square)

                # pass 1b: mm2, packed 4-way by column strips
                # (tile_position col-tiling): zz partition 32c+i = image i /
                # quarter c, cols 0:248 = num, cols 248:496 = 2*den
                # separate PSUM tiles so the den copy depends only on the
                # den matmuls (whole-tile deps), not on the num matmuls
                zzd = ppool.tile([128, QW], f32, tag="zzd")
                zzn = ppool.tile([128, QW], f32, tag="zzn")
                for c in range(4):
                    for s, (dh, dw) in enumerate(SHIFTS):
                        base = QW * c + 32 * dh + dw
                        nc.tensor.matmul(
                            zzd[32 * c:32 * c + 32, :],
                            lhsT=P[:, 544:576],
                            rhs=Psq[:, base:base + QW],
                            start=(s == 0), stop=(s == 3),
                            tile_position=(0, 32 * c),
                        )
                for c in range(4):
                    nc.tensor.matmul(
                        zzn[32 * c:32 * c + 32, :], lhsT=P[:, 512:544],
                        rhs=qsq[:, QW * c:QW * c + QW],
                        start=True, stop=True, tile_position=(0, 32 * c),
                    )

                # pass 2: finals.  TT-divide is not a valid DVE instruction
                # on trn2, so: reciprocal of the den half (copied out early —
                # it only depends on the den matmuls, so recip runs hidden
                # under the num matmuls), then copy(num) -> multiply.
                nc.scalar.copy(zs[:, QW:2 * QW], zzd[:])
                rden = wpool.tile([128, QW], bf16, tag="rden")
                with nc.allow_low_precision("bf16 ok at 2e-2 tolerance"):
                    nc.vector.reciprocal(rden[:], zs[:, QW:2 * QW])
                nc.scalar.copy(zs[:, 0:QW], zzn[:])
                nc.vector.tensor_tensor(
                    res[:, 0:QW], zs[:, 0:QW], rden[:], ALU.mult,
                )
                nc.sync.dma_start(out=out[:, :], in_=res[:])

            if loop_reps is None:
                warmup()
                load()
                body()
            else:
                warmup()
                with tc.For_i(0, loop_reps, 1):
                    for _ in range(loop_unroll):
                        load()
                        body()
    nc.compile()
    return nc


def _get_bass():
    if "nc" not in _CACHE:
        _CACHE["nc"] = _build_bass()
    return _CACHE["nc"]


def _prep_inputs(img, weights_mps):
    img = np.ascontiguousarray(np.asarray(img, dtype=np.float32))
    U = _build_U(weights_mps)
    A = U[:4, :]

    Wt = np.zeros((128, PCOL0), dtype=np.float32)
    idx = np.arange(NI)
    for s, (dh, dw) in enumerate(SHIFTS):
        for pi in range(2):
            for pj in range(2):
                pl = 2 * pi + pj
                t = 4 * (2 * dh + pi) + (2 * dw + pj)
                for j in range(4):
                    Wt[4 * idx + pl, 128 * s + 4 * idx + j] = A[j, t]
    for j in range(4):
        Wt[4 * idx + j, 512 + idx] = 1.0       # num = sum_j q_j^2
    for pl in range(4):
        Wt[4 * idx + pl, 544 + idx] = 2.0      # 2*den = 2 sum planes^2

    # parity planes: [core, 128 = img*4 + (2pi+pj), 1024 = 32r + c]
    Pl = img[:, 0].reshape(BS, 32, 2, 32, 2)          # (b, r, pi, c, pj)
    Pl = Pl.transpose(0, 2, 4, 1, 3).reshape(N_CORES, 128, 1024)

    blob = np.zeros((N_CORES, 128, PCOL0 + PCOLS), dtype=np.float32)
    blob[:, :, 0:PCOL0] = Wt[None]
    blob[:, :, PCOL0:PCOL0 + 1024] = Pl
    blob16 = blob.astype(ml_dtypes.bfloat16)
    return (
        np.ascontiguousarray(blob16[:, :, 0:D0]),
        np.ascontiguousarray(blob16[:, :, D0:]),
    )


def kernel(img: np.ndarray, weights_mps: np.ndarray) -> np.ndarray:
    from concourse.bass_utils import run_bass_kernel_spmd

    blob0, blob1 = _prep_inputs(img, weights_mps)
    nc = _get_bass()
    in_maps = [{"blob0": blob0[c], "blob1": blob1[c]} for c in range(N_CORES)]
    r = run_bass_kernel_spmd(
        nc, in_maps, list(range(N_CORES)), trace=TRACE, **TRACE_KWARGS
    )
    if TRACE:
        _CACHE["last_result"] = r

    outs = np.stack([np.asarray(r.results[c]["out"]) for c in range(N_CORES)])
    # [core, 32c+i, k] -> image core*32+i, pixel-buffer col 248c+k
    res = (
        outs.astype(np.float32)[:, :, 0:QW]
        .reshape(N_CORES, 4, NI, QW)
        .transpose(0, 2, 1, 3)
        .reshape(BS, 31, 32)[:, :, :31]
    ) + np.float32(0.5)          # device ships num/(2den); host adds the 0.5
    return np.ascontiguousarray(res.reshape(BS, 1, OH * OW))
